# revision 2
# baseline (speedup 1.0000x reference)
"""Trainium2 Bass kernel V3 for multiplicative-tril-mask attention (8 cores).

Problem: B=4, T=2048, DIN=DOUT=1024
  q = x @ Wq.T ; k = x @ Wk.T ; v = x @ Wv.T
  attn = (q @ k.T) * tril_ones        # multiplicative mask: masked logits -> 0
  attn = softmax(attn / sqrt(T))      # masked entries contribute exp(0)=1
  out = attn @ v

V3 structural wins over V2 (152 us):
 1. fp8 DoubleRow yT projection: M and xq are host-cast to e4m3 with a
    x8 extra scale (YSC2=512) so M rms ~0.12 and y rms ~3.8 sit in e4m3's
    normal range; halves the yT tensor time (27.3 -> 13.6 us).
 2. Host-swizzled inputs: every DRAM tensor is stored [128, nt, W] so a
    single DMA instruction with 1-16KB per-partition contiguous runs loads
    a whole tensor half (HWDGE fixed cost is 625ns/instruction, so V2's
    ~60 input DMAs were ~40us of queue time; V3 uses 12).
 3. gpsimd queue carries ONLY the V-bounce writes + AllGather doorbells
    (V2 had 30+ DMAs serialized ahead of the AG triggers, delaying them
    to 53/64us). Plus a dummy warmup AllGather at t~8us to absorb the
    ~15us CC-stream bootstrap barrier.
 4. PE-clock warmup matmuls during the initial DMA window (PE ramps
    0.65->2.4GHz with activity; V2's first ~25 matmuls ran at 1.2GHz).
 5. PV emits per-o [128,1024] outputs (one DMA each, descending o so the
    final output DMA is the cheapest chain).
"""

import os
import sys

sys.path.insert(0, "/opt/trn_rl_repo")

import numpy as np
import ml_dtypes

import concourse.bass as bass
import concourse.tile as tile
from concourse import bacc, mybir
from concourse import bass_utils

bass_utils.upload_artifacts = lambda tmpdir: "local://" + tmpdir

B, T, D = 4, 2048, 1024
N_CORES = 8
NDT = D // 128          # 8 contraction tiles
NKT_ALL = T // 128      # 16 key tiles
HALF = T // 2           # 1024

SCALE = 1.0 / float(np.sqrt(np.float32(T)))
GROUPS = [[0, 1], [2, 3], [4, 5], [6, 7]]

BF = mybir.dt.bfloat16
F8 = mybir.dt.float8e4
F32 = mybir.dt.float32
bf16 = ml_dtypes.bfloat16
f8e4 = ml_dtypes.float8_e4m3

# fp8 DoubleRow scores AND yT projection. y is scaled by YSC2 (folded into
# M on the host) so both M (~0.12 rms) and y (~3.8 rms) sit in e4m3's
# normal range; the exp activation divides the scale back out.
YSC2 = 512.0

_cache = {}
LAST_RESULT = None


def _w(o):          # PV window (k-tiles) for owned subtile slot o
    return 2 * o + 2


def _wsc(g):        # score window (k-tiles) for score group g
    return 4 * g + 4


def _build():
    nc = bacc.Bacc("TRN2", target_bir_lowering=False, debug=False, num_devices=N_CORES)

    # All inputs are host-swizzled to [128, nt, W]: partition-major with
    # per-partition contiguous runs, so one DMA instruction covers a
    # whole tensor (or a column half for pipelining).
    xT_d = nc.dram_tensor("xT", [128, NDT, T], F8, kind="ExternalInput")
    xTh_d = nc.dram_tensor("xTh", [128, NDT, HALF], BF, kind="ExternalInput")
    xTq_d = nc.dram_tensor("xTq", [128, NDT, 1024], F8, kind="ExternalInput")
    M_d = nc.dram_tensor("M", [128, NDT, D], F8, kind="ExternalInput")
    wv_d = nc.dram_tensor("wv", [128, NDT, D], BF, kind="ExternalInput")
    qmi_d = nc.dram_tensor("qmi", [128, 4, 512], F32, kind="ExternalInput")
    ssuf_d = nc.dram_tensor("ssuf", [1, 8 * D], BF, kind="ExternalInput")
    out_d = nc.dram_tensor("out", [1024, D], BF, kind="ExternalOutput")

    xT_ap = xT_d.ap()
    xTh = xTh_d.ap()
    xTq = xTq_d.ap()
    qmi_ap = qmi_d.ap()
    out_ap = out_d.ap()

    Exp = mybir.ActivationFunctionType.Exp

    with tile.TileContext(nc) as tc:
        with (
            tc.tile_pool(name="actpool", bufs=1) as actpool,
            tc.tile_pool(name="cpool", bufs=1) as cpool,
            tc.tile_pool(name="drpool", bufs=1, space="DRAM") as drpool,
            tc.tile_pool(name="ps_big", bufs=6, space="PSUM") as ps_big,
            tc.tile_pool(name="ps_small", bufs=2, space="PSUM") as ps_small,
        ):
            # ---- constants ----
            ones_col = cpool.tile([128, 1], BF)
            nc.vector.memset(ones_col[:], 1.0)
            ones_row = cpool.tile([1, 128], BF)
            nc.vector.memset(ones_row[:], 1.0)
            one11 = cpool.tile([1, 1], F32)
            nc.vector.memset(one11[:], 1.0)
            warm = cpool.tile([128, 512], BF)
            nc.vector.memset(warm[:], 0.000488)

            qmi = cpool.tile([128, 4, 512], F32)
            ssuf = cpool.tile([1, 8 * D], BF)

            # persistent activations
            xT = actpool.tile([128, NDT, T], F8, tag="xt")
            yT = actpool.tile([128, NDT, 1024], F8, tag="yt")
            V = actpool.tile([128, NKT_ALL, D], BF, tag="v")
            Vst = [
                actpool.tile([128, NDT, 512], BF, tag=f"vst{ec}", name=f"vst{ec}")
                for ec in range(2)
            ]
            pT = [
                actpool.tile([128, _wsc(g), 256], BF, tag=f"pt{g}", name=f"pt{g}")
                for g in range(4)
            ]

            # DRAM bounce buffers for the V collective
            vbounce = [
                drpool.tile([128, 4 * D], BF, name=f"vbounce{h}") for h in range(2)
            ]
            vg = [drpool.tile([256, 4 * D], BF, name=f"vg{h}") for h in range(2)]
            # tiny scratch for the CC-stream warmup collective
            cwarm_in = drpool.tile([1, 64], BF, name="cwarm_in")
            cwarm_out = drpool.tile([2, 64], BF, name="cwarm_out")

            with (
                tc.tile_pool(name="xpool", bufs=1) as xpool,
                tc.tile_pool(name="wpool", bufs=1) as wpool,
                tc.tile_pool(name="mpool", bufs=3) as mpool,
                tc.tile_pool(name="spool", bufs=2) as spool,
                tc.tile_pool(name="opool", bufs=3) as opool,
            ):
                wv_t = wpool.tile([128, NDT, D], BF, tag="wv")
                Mt = wpool.tile([128, NDT, D], F8, tag="m")
                xh_t = xpool.tile([128, NDT, HALF], BF, tag="xh")
                xq_t = xpool.tile([128, NDT, 1024], F8, tag="xq")

                # ---- input DMAs: one instruction per tensor column-half ----
                # sync queue: V-proj weights first, then scores inputs
                nc.sync.dma_start(wv_t[:, :, 0:512], wv_d.ap()[:, :, 0:512])
                nc.sync.dma_start(wv_t[:, :, 512:1024], wv_d.ap()[:, :, 512:1024])
                nc.sync.dma_start(xq_t[:, :, 0:512], xTq[:, :, 0:512])
                nc.sync.dma_start(xT[:, :, 0:1024], xT_ap[:, :, 0:1024])
                nc.sync.dma_start(xq_t[:, :, 512:1024], xTq[:, :, 512:1024])
                nc.sync.dma_start(xT[:, :, 1024:2048], xT_ap[:, :, 1024:2048])
                # scalar queue: V-proj activations first, then yT inputs
                nc.scalar.dma_start(xh_t[:, :, 0:512], xTh[:, :, 0:512])
                nc.scalar.dma_start(xh_t[:, :, 512:1024], xTh[:, :, 512:1024])
                nc.scalar.dma_start(Mt[:, :, 0:512], M_d.ap()[:, :, 0:512])
                nc.scalar.dma_start(Mt[:, :, 512:1024], M_d.ap()[:, :, 512:1024])
                nc.scalar.dma_start(qmi[:, :, :], qmi_ap[:, :, :])
                nc.scalar.dma_start(ssuf[:], ssuf_d.ap())

                # CC-stream warmup: absorb the bootstrap barrier (~15us) while
                # input DMAs stream; the real AllGathers then start instantly.
                nc.gpsimd.collective_compute(
                    "AllGather",
                    mybir.AluOpType.bypass,
                    replica_groups=GROUPS,
                    ins=[cwarm_in.opt()],
                    outs=[cwarm_out.opt()],
                )

                # PE-clock warmup: the PE ramps 0.65->2.4GHz with activity;
                # run throwaway matmuls on memset data during the DMA window.
                for wi in range(10):
                    wps = ps_big.tile([128, 512], F32, tag="big", name="wps")
                    for wj in range(4):
                        nc.tensor.matmul(
                            wps[:],
                            warm[:, 0:128],
                            warm[:],
                            start=(wj == 0),
                            stop=(wj == 3),
                        )

                # ---- phase A: V projection (own half) + paired exchange ----
                def v_chain(ec, i):
                    ps = ps_big.tile([128, 512], F32, tag="big", name="ps")
                    for dt in range(NDT):
                        nc.tensor.matmul(
                            ps[:],
                            xh_t[:, dt, 128 * i : 128 * (i + 1)],
                            wv_t[:, dt, 512 * ec : 512 * (ec + 1)],
                            start=(dt == 0),
                            stop=(dt == NDT - 1),
                        )
                    nc.vector.tensor_copy(Vst[ec][:, i, :], ps[:])

                for ec in range(2):
                    for i in range(8):
                        v_chain(ec, i)

                # gpsimd: ONLY bounce writes + AG doorbells (everything else
                # would serialize ahead of the collective triggers)
                for ec in range(2):
                    for h2 in range(2):
                        nc.gpsimd.dma_start(
                            vbounce[ec][:, 2048 * h2 : 2048 * (h2 + 1)],
                            Vst[ec][:, 4 * h2 : 4 * (h2 + 1), :],
                        )
                    nc.gpsimd.collective_compute(
                        "AllGather",
                        mybir.AluOpType.bypass,
                        replica_groups=GROUPS,
                        ins=[vbounce[ec].opt()],
                        outs=[vg[ec].opt()],
                    )
                # readback gathered V on sync (idle after input loads)
                for ec in range(2):
                    for h in range(2):
                        nc.sync.dma_start(
                            V[:, 8 * h : 8 * (h + 1), 512 * ec : 512 * (ec + 1)],
                            vg[ec][128 * h : 128 * (h + 1), :],
                        )

                # ---- yT = M^T x (fp8 DoubleRow), c-major halves ----
                def yt_half(c):
                    for et in range(NDT):
                        ps = ps_big.tile([128, 512], F32, tag="big", name="ps")
                        for d2 in range(NDT // 2):
                            nc.tensor.matmul(
                                ps[:],
                                Mt[:, 2 * d2 : 2 * d2 + 2, 128 * et : 128 * (et + 1)],
                                xq_t[:, 2 * d2 : 2 * d2 + 2, 512 * c : 512 * (c + 1)],
                                start=(d2 == 0),
                                stop=(d2 == NDT // 2 - 1),
                                perf_mode=mybir.MatmulPerfMode.DoubleRow,
                            )
                        nc.vector.tensor_copy(yT[:, et, 512 * c : 512 * (c + 1)], ps[:])

                # ---- phase B: scores (grouped), denominators, PV ----
                rcols = {}

                def scores_group(g):
                    for kt in range(_wsc(g)):
                        zpsA = ps_big.tile([128, 512], F32, tag="big", name="zps")
                        zps = zpsA[:, 0:256]
                        for d2 in range(NDT // 2):
                            nc.tensor.matmul(
                                zps,
                                xT[:, 2 * d2 : 2 * d2 + 2, 128 * kt : 128 * (kt + 1)],
                                yT[:, 2 * d2 : 2 * d2 + 2, 256 * g : 256 * (g + 1)],
                                start=(d2 == 0),
                                stop=(d2 == NDT // 2 - 1),
                                perf_mode=mybir.MatmulPerfMode.DoubleRow,
                            )
                        if kt >= 4 * g:
                            mt = mpool.tile([128, 256], F32, tag="mask", name="mt")
                            nc.vector.tensor_scalar(
                                mt[:],
                                qmi[:, g, 0:256],
                                float(128 * kt),
                                None,
                                op0=mybir.AluOpType.is_ge,
                            )
                            nc.vector.tensor_mul(zps, zps, mt[:])
                        nc.scalar.activation(
                            pT[g][:, kt, :],
                            zps,
                            Exp,
                            scale=1.0 / YSC2,
                        )

                def den_rcol(o):
                    g, c = o // 2, o % 2
                    w = _w(o)
                    dps = ps_small.tile([1, 512], F32, tag="small", name="dps", bufs=1)
                    chunks = [(s, min(4, w - s)) for s in range(0, w, 4)]
                    for ci, (s, nk) in enumerate(chunks):
                        nc.tensor.matmul(
                            dps[0:1, 0 : 128 * nk],
                            ones_col[:],
                            pT[g][:, s : s + nk, 128 * c : 128 * (c + 1)],
                            start=(ci == 0),
                            stop=(ci == len(chunks) - 1),
                        )
                    nv = 128 * min(w, 4)
                    dsb = spool.tile([1, 512], F32, tag="dsb", name="dsb")
                    nc.vector.tensor_copy(dsb[0:1, 0:nv], dps[0:1, 0:nv])
                    t2 = spool.tile([1, 128], F32, tag="t2", name="t2")
                    if w == 2:
                        nc.vector.tensor_add(t2[:], dsb[0:1, 0:128], dsb[0:1, 128:256])
                    else:
                        t1 = spool.tile([1, 256], F32, tag="t1", name="t1")
                        nc.vector.tensor_add(t1[:], dsb[0:1, 0:256], dsb[0:1, 256:512])
                        nc.vector.tensor_add(t2[:], t1[0:1, 0:128], t1[0:1, 128:256])
                    drow = spool.tile([1, 128], F32, tag="drow", name="drow")
                    nc.vector.tensor_scalar_add(
                        drow[:], t2[:], float(T - 128 * w)
                    )
                    rrow = spool.tile([1, 128], F32, tag="rrow", name="rrow")
                    nc.vector.reciprocal(rrow[:], drow[:])
                    rps = ps_small.tile([128, 1], F32, tag="rden", name="rps", bufs=1)
                    nc.tensor.matmul(rps[:], rrow[0:1, :], one11[:])
                    rcol = spool.tile([128, 1], F32, tag="rcol", name="rcol", bufs=8)
                    nc.vector.tensor_copy(rcol[:], rps[:])
                    rcols[o] = rcol

                def pv_o(o):
                    g, c = o // 2, o % 2
                    w = _w(o)
                    ot = opool.tile([128, 1024], BF, tag="out", name="ot")
                    for ec in range(2):
                        nps = ps_big.tile([128, 512], F32, tag="big", name="nps")
                        for kt in range(w):
                            nc.tensor.matmul(
                                nps[:],
                                pT[g][:, kt, 128 * c : 128 * (c + 1)],
                                V[:, kt, 512 * ec : 512 * (ec + 1)],
                                start=(kt == 0),
                                stop=(kt == w - 1 and w == NKT_ALL),
                            )
                        if w < NKT_ALL:
                            nc.tensor.matmul(
                                nps[:],
                                ones_row[:],
                                ssuf[0:1, D * o + 512 * ec : D * o + 512 * (ec + 1)],
                                start=False,
                                stop=True,
                            )
                        nc.vector.tensor_scalar_mul(
                            ot[:, 512 * ec : 512 * (ec + 1)], nps[:], rcols[o][:]
                        )
                    nc.scalar.dma_start(
                        out_ap[128 * o : 128 * (o + 1), :],
                        ot[:],
                    )

                yt_half(0)
                scores_group(0)
                den_rcol(0)
                den_rcol(1)
                scores_group(1)
                den_rcol(2)
                den_rcol(3)
                yt_half(1)
                scores_group(2)
                den_rcol(4)
                den_rcol(5)
                scores_group(3)
                den_rcol(7)
                den_rcol(6)
                for o in range(7, -1, -1):
                    pv_o(o)

    nc.compile()
    return nc


def get_nc():
    if "nc" not in _cache:
        _cache["nc"] = _build()
    return _cache["nc"]


def _swz(a, nt, w, dty):
    """[nt*128, w] row-major -> [128, nt, w] partition-major swizzle."""
    return np.ascontiguousarray(
        a.reshape(nt, 128, w).transpose(1, 0, 2)
    ).astype(dty)


def make_in_maps(x, Wq, Wk, Wv):
    x = np.asarray(x, np.float32)
    Wq32 = np.asarray(Wq, np.float32)
    Wk32 = np.asarray(Wk, np.float32)
    Wv32 = np.asarray(Wv, np.float32)

    msc = SCALE * YSC2
    M_sw = _swz((Wq32.T @ Wk32) * np.float32(msc), NDT, D, f8e4)
    wv_sw = _swz(np.ascontiguousarray(Wv32.T), NDT, D, bf16)

    # qmi per parity: [128, 4, 512] f32; columns [0:256] hold q_glob - k,
    # columns [256:512] hold q_glob - k - 128 (the odd kt of the pair)
    qmis = []
    for p in range(2):
        q = np.empty((4, 128, 512), np.float32)
        for g in range(4):
            for half in range(2):
                sub = 4 * g + 2 * half + p
                q[g, :, 128 * half : 128 * (half + 1)] = (
                    128 * sub + np.arange(128, dtype=np.float32)
                )[None, :] - np.arange(128, dtype=np.float32)[:, None]
            q[g, :, 256:512] = q[g, :, 0:256] - 128.0
        qmis.append(np.ascontiguousarray(q.transpose(1, 0, 2)))

    # ssuf per batch: row o = colsum_{k >= 256(o+1)} V  (o=7 -> zeros)
    ssufs = []
    for b in range(B):
        rows = np.zeros((8, D), np.float32)
        for o in range(7):
            cs = x[b][256 * (o + 1) :, :].sum(axis=0, dtype=np.float32)
            rows[o] = cs @ Wv32.T
        ssufs.append(rows.reshape(1, 8 * D).astype(bf16))

    in_maps = []
    for core in range(N_CORES):
        b, p = core // 2, core % 2
        xt32 = np.ascontiguousarray(x[b].T)  # [D, T] f32
        xt = _swz(xt32, NDT, T, f8e4)
        xh = _swz(
            np.ascontiguousarray(xt32[:, HALF * p : HALF * (p + 1)]), NDT, HALF, bf16
        )
        cols = [
            xt32[:, 128 * (2 * o + p) : 128 * (2 * o + p) + 128] for o in range(8)
        ]
        xq = _swz(np.ascontiguousarray(np.concatenate(cols, axis=1)), NDT, 1024, f8e4)
        in_maps.append(
            {
                "xT": xt,
                "xTh": xh,
                "xTq": xq,
                "M": M_sw,
                "wv": wv_sw,
                "qmi": qmis[p],
                "ssuf": ssufs[b],
            }
        )
    return in_maps


def assemble(results):
    full = np.empty((B, T, D), np.float32)
    for core in range(N_CORES):
        b, p = core // 2, core % 2
        o_np = np.asarray(results[core]["out"], dtype=np.float32)
        for o in range(8):
            g = 2 * o + p
            full[b, 128 * g : 128 * (g + 1), :] = o_np[128 * o : 128 * (o + 1), :]
    return full


def kernel(x, Wq, Wk, Wv):
    global LAST_RESULT
    nc = get_nc()
    in_maps = make_in_maps(x, Wq, Wk, Wv)
    res = bass_utils.run_bass_kernel_spmd(nc, in_maps, core_ids=list(range(N_CORES)))
    LAST_RESULT = res
    return assemble(res.results)


# revision 3
# speedup vs baseline: 1.0293x; 1.0293x over previous
"""Trainium2 Bass kernel V3 for multiplicative-tril-mask attention (8 cores).

Problem: B=4, T=2048, DIN=DOUT=1024
  q = x @ Wq.T ; k = x @ Wk.T ; v = x @ Wv.T
  attn = (q @ k.T) * tril_ones        # multiplicative mask: masked logits -> 0
  attn = softmax(attn / sqrt(T))      # masked entries contribute exp(0)=1
  out = attn @ v

V3 structural wins over V2 (152 us):
 1. fp8 DoubleRow yT projection: M and xq are host-cast to e4m3 with a
    x8 extra scale (YSC2=512) so M rms ~0.12 and y rms ~3.8 sit in e4m3's
    normal range; halves the yT tensor time (27.3 -> 13.6 us).
 2. Host-swizzled inputs: every DRAM tensor is stored [128, nt, W] so a
    single DMA instruction with 1-16KB per-partition contiguous runs loads
    a whole tensor half (HWDGE fixed cost is 625ns/instruction, so V2's
    ~60 input DMAs were ~40us of queue time; V3 uses 12).
 3. gpsimd queue carries ONLY the V-bounce writes + AllGather doorbells
    (V2 had 30+ DMAs serialized ahead of the AG triggers, delaying them
    to 53/64us). Plus a dummy warmup AllGather at t~8us to absorb the
    ~15us CC-stream bootstrap barrier.
 4. PE-clock warmup matmuls during the initial DMA window (PE ramps
    0.65->2.4GHz with activity; V2's first ~25 matmuls ran at 1.2GHz).
 5. PV emits per-o [128,1024] outputs (one DMA each, descending o so the
    final output DMA is the cheapest chain).
"""

import os
import sys

sys.path.insert(0, "/opt/trn_rl_repo")

import numpy as np
import ml_dtypes

import concourse.bass as bass
import concourse.tile as tile
from concourse import bacc, mybir
from concourse import bass_utils

bass_utils.upload_artifacts = lambda tmpdir: "local://" + tmpdir

B, T, D = 4, 2048, 1024
N_CORES = 8
NDT = D // 128          # 8 contraction tiles
NKT_ALL = T // 128      # 16 key tiles
HALF = T // 2           # 1024

SCALE = 1.0 / float(np.sqrt(np.float32(T)))
GROUPS = [[0, 1], [2, 3], [4, 5], [6, 7]]

BF = mybir.dt.bfloat16
F8 = mybir.dt.float8e4
F32 = mybir.dt.float32
bf16 = ml_dtypes.bfloat16
f8e4 = ml_dtypes.float8_e4m3

# fp8 DoubleRow scores AND yT projection. y is scaled by YSC2 (folded into
# M on the host) so both M (~0.12 rms) and y (~3.8 rms) sit in e4m3's
# normal range; the exp activation divides the scale back out.
YSC2 = 512.0

_cache = {}
LAST_RESULT = None


def _w(o):          # PV window (k-tiles) for owned subtile slot o
    return 2 * o + 2


def _wsc(g):        # score window (k-tiles) for score group g
    return 4 * g + 4


def _build():
    nc = bacc.Bacc("TRN2", target_bir_lowering=False, debug=False, num_devices=N_CORES)

    # All inputs are host-swizzled to [128, nt, W]: partition-major with
    # per-partition contiguous runs, so one DMA instruction covers a
    # whole tensor (or a column half for pipelining).
    xT_d = nc.dram_tensor("xT", [128, NDT, T], F8, kind="ExternalInput")
    xTh_d = nc.dram_tensor("xTh", [128, NDT, HALF], BF, kind="ExternalInput")
    xTq_d = nc.dram_tensor("xTq", [128, NDT, 1024], F8, kind="ExternalInput")
    M_d = nc.dram_tensor("M", [128, NDT, D], F8, kind="ExternalInput")
    wv_d = nc.dram_tensor("wv", [128, NDT, D], BF, kind="ExternalInput")
    qmi_d = nc.dram_tensor("qmi", [128, 4, 512], F32, kind="ExternalInput")
    ssuf_d = nc.dram_tensor("ssuf", [1, 8 * D], BF, kind="ExternalInput")
    out_d = nc.dram_tensor("out", [1024, D], BF, kind="ExternalOutput")

    xT_ap = xT_d.ap()
    xTh = xTh_d.ap()
    xTq = xTq_d.ap()
    qmi_ap = qmi_d.ap()
    out_ap = out_d.ap()

    Exp = mybir.ActivationFunctionType.Exp

    with tile.TileContext(nc) as tc:
        with (
            tc.tile_pool(name="actpool", bufs=1) as actpool,
            tc.tile_pool(name="cpool", bufs=1) as cpool,
            tc.tile_pool(name="drpool", bufs=1, space="DRAM") as drpool,
            tc.tile_pool(name="ps_big", bufs=6, space="PSUM") as ps_big,
            tc.tile_pool(name="ps_small", bufs=2, space="PSUM") as ps_small,
        ):
            # ---- constants ----
            ones_col = cpool.tile([128, 1], BF)
            nc.vector.memset(ones_col[:], 1.0)
            ones_row = cpool.tile([1, 128], BF)
            nc.vector.memset(ones_row[:], 1.0)
            one11 = cpool.tile([1, 1], F32)
            nc.vector.memset(one11[:], 1.0)
            warm = cpool.tile([128, 512], BF)
            nc.vector.memset(warm[:], 0.000488)

            qmi = cpool.tile([128, 4, 512], F32)
            ssuf = cpool.tile([1, 8 * D], BF)

            # persistent activations
            xT = actpool.tile([128, NDT, T], F8, tag="xt")
            yT = actpool.tile([128, NDT, 1024], F8, tag="yt")
            V = actpool.tile([128, NKT_ALL, D], BF, tag="v")
            Vst = [
                actpool.tile([128, NDT, 512], BF, tag=f"vst{ec}", name=f"vst{ec}")
                for ec in range(2)
            ]
            pT = [
                actpool.tile([128, _wsc(g), 256], BF, tag=f"pt{g}", name=f"pt{g}")
                for g in range(4)
            ]

            # DRAM bounce buffers for the V collective
            vbounce = [
                drpool.tile([128, 4 * D], BF, name=f"vbounce{h}") for h in range(2)
            ]
            vg = [drpool.tile([256, 4 * D], BF, name=f"vg{h}") for h in range(2)]
            # tiny scratch for the CC-stream warmup collective
            cwarm_in = drpool.tile([1, 64], BF, name="cwarm_in")
            cwarm_out = drpool.tile([2, 64], BF, name="cwarm_out")

            with (
                tc.tile_pool(name="xpool", bufs=1) as xpool,
                tc.tile_pool(name="wpool", bufs=1) as wpool,
                tc.tile_pool(name="mpool", bufs=3) as mpool,
                tc.tile_pool(name="spool", bufs=2) as spool,
                tc.tile_pool(name="opool", bufs=3) as opool,
            ):
                wv_t = wpool.tile([128, NDT, D], BF, tag="wv")
                Mt = wpool.tile([128, NDT, D], F8, tag="m")
                xh_t = xpool.tile([128, NDT, HALF], BF, tag="xh")
                xq_t = xpool.tile([128, NDT, 1024], F8, tag="xq")

                # ---- input DMAs: one instruction per tensor column-half ----
                # sync queue: V-proj weights first, then scores inputs
                nc.sync.dma_start(wv_t[:, :, 0:512], wv_d.ap()[:, :, 0:512])
                nc.sync.dma_start(wv_t[:, :, 512:1024], wv_d.ap()[:, :, 512:1024])
                nc.sync.dma_start(xq_t[:, :, 0:512], xTq[:, :, 0:512])
                nc.sync.dma_start(xT[:, :, 0:1024], xT_ap[:, :, 0:1024])
                nc.sync.dma_start(xq_t[:, :, 512:1024], xTq[:, :, 512:1024])
                nc.sync.dma_start(xT[:, :, 1024:2048], xT_ap[:, :, 1024:2048])
                # scalar queue: V-proj activations first, then yT inputs
                nc.scalar.dma_start(xh_t[:, :, 0:512], xTh[:, :, 0:512])
                nc.scalar.dma_start(xh_t[:, :, 512:1024], xTh[:, :, 512:1024])
                nc.scalar.dma_start(Mt[:, :, 0:512], M_d.ap()[:, :, 0:512])
                nc.scalar.dma_start(Mt[:, :, 512:1024], M_d.ap()[:, :, 512:1024])

                # CC-stream warmup: absorb the bootstrap barrier (~15us) while
                # input DMAs stream; the real AllGathers then start instantly.
                nc.gpsimd.collective_compute(
                    "AllGather",
                    mybir.AluOpType.bypass,
                    replica_groups=GROUPS,
                    ins=[cwarm_in.opt()],
                    outs=[cwarm_out.opt()],
                )
                # small, late-needed loads go on the otherwise-idle gpsimd
                # queue so the mask ops the scheduler hoists between the Vst
                # copies unblock early (qmi landing late stalls the V pipe)
                nc.gpsimd.dma_start(qmi[:, :, :], qmi_ap[:, :, :])
                nc.gpsimd.dma_start(ssuf[:], ssuf_d.ap())

                # PE-clock warmup: the PE ramps 0.65->2.4GHz with activity;
                # a few throwaway matmuls during the DMA window start the
                # ramp (kept tiny: they serialize ahead of the real chains).
                for wi in range(6):
                    wps = ps_big.tile([128, 512], F32, tag="big", name="wps")
                    nc.tensor.matmul(
                        wps[:], warm[:, 0:128], warm[:], start=True, stop=True
                    )

                # ---- phase A: V projection (own half) + paired exchange ----
                def v_chain(ec, i):
                    ps = ps_big.tile([128, 512], F32, tag="big", name="ps")
                    for dt in range(NDT):
                        nc.tensor.matmul(
                            ps[:],
                            xh_t[:, dt, 128 * i : 128 * (i + 1)],
                            wv_t[:, dt, 512 * ec : 512 * (ec + 1)],
                            start=(dt == 0),
                            stop=(dt == NDT - 1),
                        )
                    nc.vector.tensor_copy(Vst[ec][:, i, :], ps[:])

                for ec in range(2):
                    for i in range(8):
                        v_chain(ec, i)

                # gpsimd: ONLY bounce writes + AG doorbells (everything else
                # would serialize ahead of the collective triggers)
                for ec in range(2):
                    for h2 in range(2):
                        nc.gpsimd.dma_start(
                            vbounce[ec][:, 2048 * h2 : 2048 * (h2 + 1)],
                            Vst[ec][:, 4 * h2 : 4 * (h2 + 1), :],
                        )
                    nc.gpsimd.collective_compute(
                        "AllGather",
                        mybir.AluOpType.bypass,
                        replica_groups=GROUPS,
                        ins=[vbounce[ec].opt()],
                        outs=[vg[ec].opt()],
                    )
                # readback gathered V on sync (idle after input loads)
                for ec in range(2):
                    for h in range(2):
                        nc.sync.dma_start(
                            V[:, 8 * h : 8 * (h + 1), 512 * ec : 512 * (ec + 1)],
                            vg[ec][128 * h : 128 * (h + 1), :],
                        )

                # ---- yT = M^T x (fp8 DoubleRow), c-major halves ----
                def yt_half(c):
                    for et in range(NDT):
                        ps = ps_big.tile([128, 512], F32, tag="big", name="ps")
                        for d2 in range(NDT // 2):
                            nc.tensor.matmul(
                                ps[:],
                                Mt[:, 2 * d2 : 2 * d2 + 2, 128 * et : 128 * (et + 1)],
                                xq_t[:, 2 * d2 : 2 * d2 + 2, 512 * c : 512 * (c + 1)],
                                start=(d2 == 0),
                                stop=(d2 == NDT // 2 - 1),
                                perf_mode=mybir.MatmulPerfMode.DoubleRow,
                            )
                        nc.vector.tensor_copy(yT[:, et, 512 * c : 512 * (c + 1)], ps[:])

                # ---- phase B: scores (grouped), denominators, PV ----
                rcols = {}

                def scores_group(g):
                    for kt in range(_wsc(g)):
                        zpsA = ps_big.tile([128, 512], F32, tag="big", name="zps")
                        zps = zpsA[:, 0:256]
                        for d2 in range(NDT // 2):
                            nc.tensor.matmul(
                                zps,
                                xT[:, 2 * d2 : 2 * d2 + 2, 128 * kt : 128 * (kt + 1)],
                                yT[:, 2 * d2 : 2 * d2 + 2, 256 * g : 256 * (g + 1)],
                                start=(d2 == 0),
                                stop=(d2 == NDT // 2 - 1),
                                perf_mode=mybir.MatmulPerfMode.DoubleRow,
                            )
                        if kt >= 4 * g:
                            mt = mpool.tile([128, 256], F32, tag="mask", name="mt")
                            nc.vector.tensor_scalar(
                                mt[:],
                                qmi[:, g, 0:256],
                                float(128 * kt),
                                None,
                                op0=mybir.AluOpType.is_ge,
                            )
                            nc.vector.tensor_mul(zps, zps, mt[:])
                        nc.scalar.activation(
                            pT[g][:, kt, :],
                            zps,
                            Exp,
                            scale=1.0 / YSC2,
                        )

                def den_rcol(o):
                    g, c = o // 2, o % 2
                    w = _w(o)
                    dps = ps_small.tile([1, 512], F32, tag="small", name="dps", bufs=1)
                    chunks = [(s, min(4, w - s)) for s in range(0, w, 4)]
                    for ci, (s, nk) in enumerate(chunks):
                        nc.tensor.matmul(
                            dps[0:1, 0 : 128 * nk],
                            ones_col[:],
                            pT[g][:, s : s + nk, 128 * c : 128 * (c + 1)],
                            start=(ci == 0),
                            stop=(ci == len(chunks) - 1),
                        )
                    nv = 128 * min(w, 4)
                    dsb = spool.tile([1, 512], F32, tag="dsb", name="dsb")
                    nc.vector.tensor_copy(dsb[0:1, 0:nv], dps[0:1, 0:nv])
                    t2 = spool.tile([1, 128], F32, tag="t2", name="t2")
                    if w == 2:
                        nc.vector.tensor_add(t2[:], dsb[0:1, 0:128], dsb[0:1, 128:256])
                    else:
                        t1 = spool.tile([1, 256], F32, tag="t1", name="t1")
                        nc.vector.tensor_add(t1[:], dsb[0:1, 0:256], dsb[0:1, 256:512])
                        nc.vector.tensor_add(t2[:], t1[0:1, 0:128], t1[0:1, 128:256])
                    drow = spool.tile([1, 128], F32, tag="drow", name="drow")
                    nc.vector.tensor_scalar_add(
                        drow[:], t2[:], float(T - 128 * w)
                    )
                    rrow = spool.tile([1, 128], F32, tag="rrow", name="rrow")
                    nc.vector.reciprocal(rrow[:], drow[:])
                    rps = ps_small.tile([128, 1], F32, tag="rden", name="rps", bufs=1)
                    nc.tensor.matmul(rps[:], rrow[0:1, :], one11[:])
                    rcol = spool.tile([128, 1], F32, tag="rcol", name="rcol", bufs=8)
                    nc.vector.tensor_copy(rcol[:], rps[:])
                    rcols[o] = rcol

                def pv_o(o):
                    g, c = o // 2, o % 2
                    w = _w(o)
                    ot = opool.tile([128, 1024], BF, tag="out", name="ot")
                    for ec in range(2):
                        nps = ps_big.tile([128, 512], F32, tag="big", name="nps")
                        for kt in range(w):
                            nc.tensor.matmul(
                                nps[:],
                                pT[g][:, kt, 128 * c : 128 * (c + 1)],
                                V[:, kt, 512 * ec : 512 * (ec + 1)],
                                start=(kt == 0),
                                stop=(kt == w - 1 and w == NKT_ALL),
                            )
                        if w < NKT_ALL:
                            nc.tensor.matmul(
                                nps[:],
                                ones_row[:],
                                ssuf[0:1, D * o + 512 * ec : D * o + 512 * (ec + 1)],
                                start=False,
                                stop=True,
                            )
                        nc.vector.tensor_scalar_mul(
                            ot[:, 512 * ec : 512 * (ec + 1)], nps[:], rcols[o][:]
                        )
                    nc.scalar.dma_start(
                        out_ap[128 * o : 128 * (o + 1), :],
                        ot[:],
                    )

                yt_half(0)
                scores_group(0)
                den_rcol(0)
                den_rcol(1)
                scores_group(1)
                den_rcol(2)
                den_rcol(3)
                yt_half(1)
                scores_group(2)
                den_rcol(4)
                den_rcol(5)
                scores_group(3)
                den_rcol(7)
                den_rcol(6)
                for o in range(7, -1, -1):
                    pv_o(o)

    nc.compile()
    return nc


def get_nc():
    if "nc" not in _cache:
        _cache["nc"] = _build()
    return _cache["nc"]


def _swz(a, nt, w, dty):
    """[nt*128, w] row-major -> [128, nt, w] partition-major swizzle."""
    return np.ascontiguousarray(
        a.reshape(nt, 128, w).transpose(1, 0, 2)
    ).astype(dty)


def make_in_maps(x, Wq, Wk, Wv):
    x = np.asarray(x, np.float32)
    Wq32 = np.asarray(Wq, np.float32)
    Wk32 = np.asarray(Wk, np.float32)
    Wv32 = np.asarray(Wv, np.float32)

    msc = SCALE * YSC2
    M_sw = _swz((Wq32.T @ Wk32) * np.float32(msc), NDT, D, f8e4)
    wv_sw = _swz(np.ascontiguousarray(Wv32.T), NDT, D, bf16)

    # qmi per parity: [128, 4, 512] f32; columns [0:256] hold q_glob - k,
    # columns [256:512] hold q_glob - k - 128 (the odd kt of the pair)
    qmis = []
    for p in range(2):
        q = np.empty((4, 128, 512), np.float32)
        for g in range(4):
            for half in range(2):
                sub = 4 * g + 2 * half + p
                q[g, :, 128 * half : 128 * (half + 1)] = (
                    128 * sub + np.arange(128, dtype=np.float32)
                )[None, :] - np.arange(128, dtype=np.float32)[:, None]
            q[g, :, 256:512] = q[g, :, 0:256] - 128.0
        qmis.append(np.ascontiguousarray(q.transpose(1, 0, 2)))

    # ssuf per batch: row o = colsum_{k >= 256(o+1)} V  (o=7 -> zeros)
    ssufs = []
    for b in range(B):
        rows = np.zeros((8, D), np.float32)
        for o in range(7):
            cs = x[b][256 * (o + 1) :, :].sum(axis=0, dtype=np.float32)
            rows[o] = cs @ Wv32.T
        ssufs.append(rows.reshape(1, 8 * D).astype(bf16))

    in_maps = []
    for core in range(N_CORES):
        b, p = core // 2, core % 2
        xt32 = np.ascontiguousarray(x[b].T)  # [D, T] f32
        xt = _swz(xt32, NDT, T, f8e4)
        xh = _swz(
            np.ascontiguousarray(xt32[:, HALF * p : HALF * (p + 1)]), NDT, HALF, bf16
        )
        cols = [
            xt32[:, 128 * (2 * o + p) : 128 * (2 * o + p) + 128] for o in range(8)
        ]
        xq = _swz(np.ascontiguousarray(np.concatenate(cols, axis=1)), NDT, 1024, f8e4)
        in_maps.append(
            {
                "xT": xt,
                "xTh": xh,
                "xTq": xq,
                "M": M_sw,
                "wv": wv_sw,
                "qmi": qmis[p],
                "ssuf": ssufs[b],
            }
        )
    return in_maps


def assemble(results):
    full = np.empty((B, T, D), np.float32)
    for core in range(N_CORES):
        b, p = core // 2, core % 2
        o_np = np.asarray(results[core]["out"], dtype=np.float32)
        for o in range(8):
            g = 2 * o + p
            full[b, 128 * g : 128 * (g + 1), :] = o_np[128 * o : 128 * (o + 1), :]
    return full


def kernel(x, Wq, Wk, Wv):
    global LAST_RESULT
    nc = get_nc()
    in_maps = make_in_maps(x, Wq, Wk, Wv)
    res = bass_utils.run_bass_kernel_spmd(nc, in_maps, core_ids=list(range(N_CORES)))
    LAST_RESULT = res
    return assemble(res.results)
